# revision 11
# baseline (speedup 1.0000x reference)
"""Trainium2 Bass kernel for relational graph convolution:

    y = sum_r (A[r] @ x) @ W[r].T        A: [8, 4096, 4096] f32
                                         x: [4096, 64] f32, W: [8, 64, 64] f32

Strategy
--------
By associativity, y = sum_r A[r] @ v_r with v_r = x @ W[r].T. Relations are
sharded across the 8 NeuronCores (expert-style parallelism); each core returns
its partial y_r.T and the host sums and transposes.

The kernel is DMA-bound: A is 64 MB/core in fp32 and must be streamed once.
To cut traffic 4x, A is shipped as float8 e3m4 (4 mantissa bits) of the
*centered and scaled* value B = (A - 0.5) * 16:
  - centering halves the magnitude (and hence the absolute quantization
    error) of the stored values; scaling by 16 lifts [-0.5, 0.5) into e3m4's
    normal range [-8, 8) (max 15.5), avoiding the flat subnormal ulp;
  - y = B.T @ (v/16) + s with s[o] = 0.5 * sum_m v[m, o], a per-partition
    bias added during the PSUM -> SBUF copy. The 1/16 is folded into W on
    the host, so the device never rescales anything.
Measured numerics (host sim, full size): rel_err ~5.5e-3 vs fp32 reference.

The TensorE contracts over the partition dim of both operands, so the host
ships A[r].T row-major; device DMAs are plain contiguous slabs.

Per core (mode "f8", the default):
  phase 1: v' = x @ (W_r.T/16) via 32 f32r matmuls (~exact at FP22), rounded
           to bf16 in SBUF [128, 32, 64]. Bias s duplicated to 128 partitions
           via one matmul of wt2 = [W'|W'] against 8*colsum(x).
  phase 2: for each 128-row contraction chunk of B.T: 8 matmuls accumulate
           y.T across 4 PSUM banks x 2 column groups of the PE array
           (tile_position col packing): group 0 (array cols 0-63) handles
           output columns n in [0, 2048) -> PSUM partitions 0-63, group 1
           (cols 64-127) handles n in [2048, 4096) -> partitions 64-127.
           The two groups stream their A halves concurrently through
           separate XBUSes, halving PE streaming time vs a 64-wide layout.
  phase 3: per-bank DVE tensor_scalar_add (+s) chases the final matmuls,
           then two 512 KB DMAs write y_r.T halves out.

Modes: "f8" (default), "f8nt" (f8 without column tiling), legacy "f32r" /
"bf16" (the previous full-width-dtype kernel).
"""

import numpy as np

import concourse.tile as tile
from concourse import bacc, mybir
from concourse.bass_utils import run_bass_kernel_spmd

R, N, IN_F, OUT_F = 8, 4096, 64, 64
P = 128            # partition dim / contraction chunk
MC = N // P        # 32 contraction chunks
BANK = 512         # fp32 elems per PSUM bank
NB = N // BANK     # 8 output column blocks
HALF = N // 2      # columns per PE column-group in mode f8
NBH = HALF // BANK # 4 banks per column-group

F32 = mybir.dt.float32
F32R = mybir.dt.float32r
BF16 = mybir.dt.bfloat16
F8E3 = mybir.dt.float8e3
F8E4 = mybir.dt.float8e4

A_SCALE = 16.0     # A shipped as fp8((A - 0.5) * A_SCALE); 1/A_SCALE folded into W
LO_SCALE = 16.0    # v residual shipped as e4m3(16 * (v' - e4m3(v'))); host folds 1/16

MODE = "f8dr"      # "f8dr" | "f8" | "f8nt" | "f32r" | "bf16"

_NC_CACHE = {}


def _build_nc_f8dr(repeat=1, jc=4, alt=True, at_bufs=4):
    """fp8(e4m3) DoubleRow kernel: 2 contraction rows per PE cell per cycle.

    A is shipped as e4m3((A.T - 0.5) * 16). The stationary operand packs
    v_hi = e4m3(v/16) into PE columns 0-63 and v_lo = e4m3(16 * residual)
    into columns 64-127, so one DoubleRow matmul stream produces both the
    main product (PSUM partitions 0-63) and the precision correction
    (partitions 64-127). ytp is [128, 4096]: rows 0-63 = hi + bias s,
    rows 64-127 = lo; the host adds hi + lo/16 across relations.

    Phase 2 is 16 double-chunks x 8 banks = 128 matmuls per pass.
    """
    assert jc % 2 == 0
    nc = bacc.Bacc("TRN2", target_bir_lowering=False, debug=False, num_devices=R)

    at = nc.dram_tensor("at", [N, N], F8E4, kind="ExternalInput").ap()
    xt = nc.dram_tensor("xt", [IN_F, N], BF16, kind="ExternalInput").ap()
    wt = nc.dram_tensor("wt", [IN_F, OUT_F], BF16, kind="ExternalInput").ap()
    xs8 = nc.dram_tensor("xs8", [IN_F, 2], BF16, kind="ExternalInput").ap()
    ytp = nc.dram_tensor("ytp", [P, N], F32, kind="ExternalOutput").ap()

    DR = mybir.MatmulPerfMode.DoubleRow
    MC2 = MC // 2   # 16 double (256-row) contraction chunks

    with tile.TileContext(nc) as tc:
        with (
            tc.tile_pool(name="const", bufs=1) as const_pool,
            tc.tile_pool(name="atp", bufs=at_bufs) as at_pool,
            tc.tile_pool(name="vp", bufs=2) as v_pool,
            tc.tile_pool(name="outp", bufs=2) as out_pool,
        ):
            xt_sb = const_pool.tile([IN_F, N], BF16)
            nc.sync.dma_start(xt_sb[:], xt[:])
            wt_sb = const_pool.tile([IN_F, OUT_F], BF16)
            nc.sync.dma_start(wt_sb[:], wt[:])
            xs8_sb = const_pool.tile([IN_F, 2], BF16)
            nc.sync.dma_start(xs8_sb[:], xs8[:])
            s_sb = const_pool.tile([OUT_F, 1], F32)

            at_r3 = at.rearrange("(c j p) n -> c p j n", p=P, j=jc)

            # phase 1: ps_v = v' = x @ (W/16).T, 16 chunks (mc) batched per
            # PSUM tile so the hi/lo split runs as 3 DVE ops per [128, 1024]
            # slab: v2[:, mc, 0:64] = e4m3(v'),
            # v2[:, mc, 64:128] = e4m3(16 * (v' - e4m3(v'))).
            MB = 16  # chunks per phase-1 batch
            v2_sb = v_pool.tile([P, MC, P], F8E4, tag="v2_sb")
            diff_sb = v_pool.tile([P, MB, OUT_F], F32, tag="diff_sb")
            with tc.tile_pool(name="psv", bufs=2, space="PSUM") as psv_pool:
                ps_s = psv_pool.tile([OUT_F, 2], F32)
                nc.tensor.matmul(ps_s[:], wt_sb[:], xs8_sb[:], start=True, stop=True)
                nc.vector.tensor_copy(s_sb[:], ps_s[:, 0:1])
                for t in range(MC // MB):
                    ps_v = psv_pool.tile([P, MB, OUT_F], F32)
                    for k in range(MB):
                        mc = t * MB + k
                        nc.tensor.matmul(
                            ps_v[:, k, :],
                            xt_sb[:, mc * P : (mc + 1) * P],
                            wt_sb[:],
                            start=True,
                            stop=True,
                        )
                    hi = v2_sb[:, t * MB : (t + 1) * MB, 0:OUT_F]
                    lo = v2_sb[:, t * MB : (t + 1) * MB, OUT_F:P]
                    nc.vector.tensor_copy(hi, ps_v[:])
                    nc.vector.tensor_tensor(
                        diff_sb[:], ps_v[:], hi, mybir.AluOpType.subtract
                    )
                    nc.vector.tensor_scalar_mul(lo, diff_sb[:], float(LO_SCALE))

            # phase 2: one DoubleRow matmul per (256-row chunk, bank):
            # lhsT [128, 2, 128], rhs [128, 2, 512] -> out [128, 512].
            with tc.tile_pool(name="psy", bufs=1, space="PSUM") as psy_pool:
                for _rep in range(repeat):
                    out_sb = out_pool.tile([P, N], F32, tag="out_sb")
                    ps_y = psy_pool.tile([P, N], F32, tag="ps_y")
                    for c in range(MC // jc):
                        at_t = at_pool.tile([P, jc, N], F8E4)
                        eng = nc.scalar if (alt and c % 2) else nc.sync
                        eng.dma_start(at_t[:], at_r3[c])
                        for j2 in range(jc // 2):
                            mc2 = c * (jc // 2) + j2
                            start = mc2 == 0
                            stop = mc2 == MC2 - 1
                            for b in range(NB):
                                fo = slice(b * BANK, (b + 1) * BANK)
                                nc.tensor.matmul(
                                    ps_y[:, fo],
                                    v2_sb[:, c * jc + 2 * j2 : c * jc + 2 * j2 + 2, :],
                                    at_t[:, 2 * j2 : 2 * j2 + 2, fo],
                                    start=start,
                                    stop=stop,
                                    perf_mode=DR,
                                )
                                # phase 3: bias-add (hi) / copy (lo) chase the
                                # final matmuls
                                if stop:
                                    nc.vector.tensor_scalar_add(
                                        out_sb[0:OUT_F, fo],
                                        ps_y[0:OUT_F, fo],
                                        s_sb[:, 0:1],
                                    )
                                    nc.vector.tensor_copy(
                                        out_sb[OUT_F:P, fo], ps_y[OUT_F:P, fo]
                                    )
                    nc.sync.dma_start(ytp[:, 0:HALF], out_sb[:, 0:HALF])
                    nc.scalar.dma_start(ytp[:, HALF:N], out_sb[:, HALF:N])

    nc.compile()
    return nc


def _build_nc_f8(repeat=1, jc=4, alt=True, at_bufs=4, coltile=True):
    """fp8(e3m4) A-streaming kernel. jc = 128-row chunks per DMA; alt =
    alternate the SP/ACT HWDGE rings between consecutive A-slab DMAs;
    coltile = pack the two output-column halves onto PE column groups."""
    nc = bacc.Bacc("TRN2", target_bir_lowering=False, debug=False, num_devices=R)

    at = nc.dram_tensor("at", [N, N], F8E3, kind="ExternalInput").ap()
    xt = nc.dram_tensor("xt", [IN_F, N], F32R, kind="ExternalInput").ap()
    wt2 = nc.dram_tensor("wt2", [IN_F, 2 * OUT_F], F32R, kind="ExternalInput").ap()
    # [IN_F, 2]: col 0 = 8*colsum(x), col 1 = 0 — fp32r matmuls need an even
    # moving free size (s3d3_mm_fp32r_restrictions)
    xs8 = nc.dram_tensor("xs8", [IN_F, 2], F32R, kind="ExternalInput").ap()
    ytp = nc.dram_tensor("ytp", [OUT_F, N], F32, kind="ExternalOutput").ap()

    with tile.TileContext(nc) as tc:
        with (
            tc.tile_pool(name="const", bufs=1) as const_pool,
            tc.tile_pool(name="atp", bufs=at_bufs) as at_pool,
            tc.tile_pool(name="vp", bufs=2) as v_pool,
            tc.tile_pool(name="outp", bufs=2) as out_pool,
        ):
            xt_sb = const_pool.tile([IN_F, N], F32R)
            nc.sync.dma_start(xt_sb[:], xt[:])
            wt2_sb = const_pool.tile([IN_F, 2 * OUT_F], F32R)
            nc.sync.dma_start(wt2_sb[:], wt2[:])
            xs8_sb = const_pool.tile([IN_F, 2], F32R)
            nc.sync.dma_start(xs8_sb[:], xs8[:])
            s_sb = const_pool.tile([P, 1], F32)

            at_r3 = at.rearrange("(c j p) n -> c p j n", p=P, j=jc)

            # phase 1: v'[m, o] = sum_i x[m, i] W'[o, i] (W' = W/16), f32r
            # (~FP22 exact), rounded to bf16 by the DVE copy. Plus the bias
            # s[o] = sum_i 8*colsum(x)[i] * W'[i, o], duplicated to both
            # partition halves via the doubled wt2.
            v_sb = v_pool.tile([P, MC, OUT_F], BF16, tag="v_sb")
            with tc.tile_pool(name="psv", bufs=2, space="PSUM") as psv_pool:
                ps_s = psv_pool.tile([P, 2], F32)
                nc.tensor.matmul(ps_s[:], wt2_sb[:], xs8_sb[:], start=True, stop=True)
                nc.vector.tensor_copy(s_sb[:], ps_s[:, 0:1])
                for mc in range(MC):
                    ps_v = psv_pool.tile([P, OUT_F], F32)
                    nc.tensor.matmul(
                        ps_v[:],
                        xt_sb[:, mc * P : (mc + 1) * P],
                        wt2_sb[:, :OUT_F],
                        start=True,
                        stop=True,
                    )
                    nc.vector.tensor_copy(v_sb[:, mc, :], ps_v[:])

            # phase 2: y.T[o, n] += sum_m v'[m, o] * B.T[m, n], bias-corrected
            # on the way out. Column-group h of the PE array handles output
            # columns [h*2048, (h+1)*2048) -> PSUM partitions [h*64, (h+1)*64).
            with tc.tile_pool(name="psy", bufs=1, space="PSUM") as psy_pool:
                for _rep in range(repeat):
                    if coltile:
                        out_sb = out_pool.tile([P, HALF], F32, tag="out_sb")
                        ps_y = psy_pool.tile([P, HALF], F32, tag="ps_y")
                    else:
                        out_sb = out_pool.tile([OUT_F, N], F32, tag="out_sb")
                        ps_y = psy_pool.tile([OUT_F, N], F32, tag="ps_y")
                    for c in range(MC // jc):
                        at_t = at_pool.tile([P, jc, N], F8E3)
                        eng = nc.scalar if (alt and c % 2) else nc.sync
                        eng.dma_start(at_t[:], at_r3[c])
                        for j in range(jc):
                            mc = c * jc + j
                            start = mc == 0
                            stop = mc == MC - 1
                            if coltile:
                                for b in range(NBH):
                                    for h in (0, 1):
                                        po = slice(h * OUT_F, (h + 1) * OUT_F)
                                        fo = slice(b * BANK, (b + 1) * BANK)
                                        nc.tensor.matmul(
                                            ps_y[po, fo],
                                            v_sb[:, mc, :],
                                            at_t[:, j, h * HALF + b * BANK : h * HALF + (b + 1) * BANK],
                                            start=start,
                                            stop=stop,
                                        )
                                        # phase 3: bias-add + copy chase the
                                        # final matmuls
                                        if stop:
                                            nc.vector.tensor_scalar_add(
                                                out_sb[po, fo],
                                                ps_y[po, fo],
                                                s_sb[po, 0:1],
                                            )
                            else:
                                for b in range(NB):
                                    fo = slice(b * BANK, (b + 1) * BANK)
                                    nc.tensor.matmul(
                                        ps_y[:, fo],
                                        v_sb[:, mc, :],
                                        at_t[:, j, fo],
                                        start=start,
                                        stop=stop,
                                    )
                                    if stop:
                                        nc.vector.tensor_scalar_add(
                                            out_sb[:, fo],
                                            ps_y[:, fo],
                                            s_sb[0:OUT_F, 0:1],
                                        )
                    if coltile:
                        nc.sync.dma_start(ytp[:, 0:HALF], out_sb[0:OUT_F, :])
                        nc.scalar.dma_start(ytp[:, HALF:N], out_sb[OUT_F:P, :])
                    else:
                        nc.sync.dma_start(ytp[:], out_sb[:])

    nc.compile()
    return nc


def _build_nc_legacy(repeat=1, mode="f32r", jc=None, alt=True, at_bufs=None):
    """Previous-generation kernel: A streamed at full dtype width (float32r
    or bf16), 64-wide stationary v, 8 PSUM banks."""
    a_dt = mybir.dt.float32r if mode == "f32r" else mybir.dt.bfloat16
    if jc is None:
        jc = 1 if mode == "f32r" else 2
    if at_bufs is None:
        at_bufs = {1: 4, 2: 3, 4: 2}[jc] if mode == "f32r" else 4

    nc = bacc.Bacc("TRN2", target_bir_lowering=False, debug=False, num_devices=R)

    at = nc.dram_tensor("at", [N, N], a_dt, kind="ExternalInput").ap()
    xt = nc.dram_tensor("xt", [IN_F, N], F32, kind="ExternalInput").ap()
    wt = nc.dram_tensor("wt", [IN_F, OUT_F], F32, kind="ExternalInput").ap()
    ytp = nc.dram_tensor("ytp", [OUT_F, N], F32, kind="ExternalOutput").ap()

    with tile.TileContext(nc) as tc:
        with (
            tc.tile_pool(name="const", bufs=1) as const_pool,
            tc.tile_pool(name="atp", bufs=at_bufs) as at_pool,
            tc.tile_pool(name="vp", bufs=2) as v_pool,
            tc.tile_pool(name="outp", bufs=2) as out_pool,
        ):
            xt_sb = const_pool.tile([IN_F, N], F32)
            nc.sync.dma_start(xt_sb[:], xt[:])
            wt_sb = const_pool.tile([IN_F, OUT_F], F32)
            nc.sync.dma_start(wt_sb[:], wt[:])

            at_r3 = at.rearrange("(c j p) n -> c p j n", p=P, j=jc)

            v_sb = v_pool.tile([P, MC, OUT_F], a_dt, tag="v_sb")
            with tc.tile_pool(name="psv", bufs=2, space="PSUM") as psv_pool:
                for mc in range(MC):
                    ps_v = psv_pool.tile([P, OUT_F], F32)
                    nc.tensor.matmul(
                        ps_v[:],
                        xt_sb[:, mc * P : (mc + 1) * P],
                        wt_sb[:],
                        start=True,
                        stop=True,
                    )
                    nc.vector.tensor_copy(v_sb[:, mc, :], ps_v[:])

            with tc.tile_pool(name="psy", bufs=1, space="PSUM") as psy_pool:
                for _rep in range(repeat):
                    out_sb = out_pool.tile([OUT_F, N], F32, tag="out_sb")
                    ps_y = psy_pool.tile([OUT_F, N], F32, tag="ps_y")
                    for c in range(MC // jc):
                        at_t = at_pool.tile([P, jc, N], a_dt)
                        eng = nc.scalar if (alt and c % 2) else nc.sync
                        eng.dma_start(at_t[:], at_r3[c])
                        for j in range(jc):
                            mc = c * jc + j
                            for b in range(NB):
                                nc.tensor.matmul(
                                    ps_y[:, b * BANK : (b + 1) * BANK],
                                    v_sb[:, mc, :],
                                    at_t[:, j, b * BANK : (b + 1) * BANK],
                                    start=(mc == 0),
                                    stop=(mc == MC - 1),
                                )
                                if mc == MC - 1:
                                    nc.vector.tensor_copy(
                                        out_sb[:, b * BANK : (b + 1) * BANK],
                                        ps_y[:, b * BANK : (b + 1) * BANK],
                                    )
                                    nc.sync.dma_start(
                                        ytp[:, b * BANK : (b + 1) * BANK],
                                        out_sb[:, b * BANK : (b + 1) * BANK],
                                    )

    nc.compile()
    return nc


def _build_nc(repeat=1, mode=None, **kwargs):
    mode = mode or MODE
    if mode == "f8dr":
        return _build_nc_f8dr(repeat, **kwargs)
    if mode == "f8":
        return _build_nc_f8(repeat, coltile=True, **kwargs)
    if mode == "f8nt":
        return _build_nc_f8(repeat, coltile=False, **kwargs)
    return _build_nc_legacy(repeat, mode=mode, **kwargs)


def make_in_maps(adjacency, x, weight, mode=None):
    mode = mode or MODE
    # Host-side layout prep: contraction dim must land on SBUF partitions.
    at_np = np.ascontiguousarray(adjacency.transpose(0, 2, 1))  # [R, m, n]
    if mode == "f8dr":
        import ml_dtypes

        at_q = ((at_np - np.float32(0.5)) * np.float32(A_SCALE)).astype(
            ml_dtypes.float8_e4m3
        )
        xt_np = np.ascontiguousarray(x.T).astype(ml_dtypes.bfloat16)
        wt_np = np.ascontiguousarray(
            weight.transpose(0, 2, 1) / np.float32(A_SCALE)
        ).astype(ml_dtypes.bfloat16)                            # [R, IN_F, OUT_F]
        xs8_np = np.zeros((IN_F, 2), dtype=ml_dtypes.bfloat16)
        xs8_np[:, 0] = (8.0 * x.sum(0, dtype=np.float64)).astype(ml_dtypes.bfloat16)
        return [
            {"at": at_q[r], "xt": xt_np, "wt": wt_np[r], "xs8": xs8_np}
            for r in range(R)
        ]
    if mode in ("f8", "f8nt"):
        import ml_dtypes

        at_q = ((at_np - np.float32(0.5)) * np.float32(A_SCALE)).astype(
            ml_dtypes.float8_e3m4
        )
        xt_np = np.ascontiguousarray(x.T)                       # [IN_F, N]
        wts = weight.transpose(0, 2, 1) / np.float32(A_SCALE)   # [R, IN_F, OUT_F]
        wt2_np = np.ascontiguousarray(np.concatenate([wts, wts], axis=2))
        xs8_np = np.zeros((IN_F, 2), dtype=np.float32)
        xs8_np[:, 0] = (8.0 * x.sum(0, dtype=np.float64)).astype(np.float32)
        return [
            {"at": at_q[r], "xt": xt_np, "wt2": wt2_np[r], "xs8": xs8_np}
            for r in range(R)
        ]
    if mode == "bf16":
        import ml_dtypes

        at_np = at_np.astype(ml_dtypes.bfloat16)
    xt_np = np.ascontiguousarray(x.T)                           # [IN_F, N]
    wt_np = np.ascontiguousarray(weight.transpose(0, 2, 1))     # [R, IN_F, OUT_F]
    return [{"at": at_np[r], "xt": xt_np, "wt": wt_np[r]} for r in range(R)]


def assemble_output(results, mode=None):
    mode = mode or MODE
    yt = np.zeros((OUT_F, N), dtype=np.float32)
    for r in range(R):
        ytp = results[r]["ytp"]
        if mode == "f8dr":
            # rows 0-63 = hi + bias, rows 64-127 = lo (scaled by LO_SCALE)
            yt += ytp[:OUT_F] + ytp[OUT_F:] * np.float32(1.0 / LO_SCALE)
        else:
            yt += ytp
    return np.ascontiguousarray(yt.T)


def run_with_results(inputs, repeat=1, mode=None):
    """Run the kernel; returns (full_output [4096, 64] f32, BassKernelResults)."""
    mode = mode or MODE
    adjacency = np.asarray(inputs["adjacency"], dtype=np.float32)
    x = np.asarray(inputs["x"], dtype=np.float32)
    weight = np.asarray(inputs["weight"], dtype=np.float32)
    assert adjacency.shape == (R, N, N)
    assert x.shape == (N, IN_F)
    assert weight.shape == (R, OUT_F, IN_F)

    in_maps = make_in_maps(adjacency, x, weight, mode)

    key = (repeat, mode)
    if key not in _NC_CACHE:
        _NC_CACHE[key] = _build_nc(repeat, mode)
    nc = _NC_CACHE[key]

    res = run_bass_kernel_spmd(nc, in_maps, core_ids=list(range(R)))
    return assemble_output(res.results, mode), res


def kernel(**inputs) -> np.ndarray:
    y, _ = run_with_results(inputs)
    return y
